# revision 19
# baseline (speedup 1.0000x reference)
"""Trainium2 Bass kernel for nn_Explainer segment_reduce (cdist + bidirectional
segment max/mean) on 8 NeuronCores.

Math (reference):
    ef_n = (h[ne0] + h[ne1])/2, ef_l = (h[le0] + h[le1])/2
    M = -cdist(ef_n, ef_l)                      # [En, El]
    out_n = seg_mean_rows(seg_max_cols(M))      # [Gn, Gl]
    out_l = seg_mean_cols(seg_max_rows(M))      # [Gn, Gl]
    out = (out_n + out_l)/2

Device computes strip = a.b - |b|^2 - |a|^2/4 = -|u_n - u_l|^2 = -4d^2 with
a = 2u_n, b = u_l (fp8): the dot via fp8 DoubleRow matmuls (K=256), BOTH
norm terms via one extra fp8 K=6 DoubleRow matmul whose rows carry a
scale-cascade fp8 decomposition (scales 8, 1, 1/8 on the opposing operand's
constant rows; |err| <= ~0.25) of -|a_r|^2/4 (stationary side) and -|b_c|^2
(moving side). All segment reductions are then plain MAX on complete
values; host maps back via d = 0.5*sqrt(-v).

Sharding: core c owns node segments [8c, 8c+8) in per-segment lane bands
(segment s -> lanes [B_s, B_s+L_s), row-tiles t in [0, nrt)); dummy slots
duplicate the segment's first row (can't win a max). Label columns
replicated; each label segment padded to a multiple of W=2 with duplicate
edges, laid out MEMBER-MAJOR: member m of block b sits at column m*B + b,
so the row-side block max is a single unit-stride tensor_max of the two
C/2 halves (full 2x DVE rate).

Engine split per tile: PE fills psum (main + bias matmul per 512-chunk);
Scalar drains five [P,1536] psum groups to the f16 strip; DVE drains the
final 544 cols from its own small psum pool (own ring -> a full tile of
slack before its release gates PE), then does the full-width col running
max and the halves row max, lagged one tile so the small drains never
queue behind them. Host: fold pair-maxes per label segment + sqrt + means;
band-collapse the col accumulator + sqrt + masked means; assemble [64,64].
"""
import numpy as np

import concourse.bacc as bacc
import concourse.tile as tile
import concourse.mybir as mybir
from concourse.bass_utils import run_bass_kernel_spmd

P = 128
N_CORES = 8
GN = GL = 64
D = 256
W = 2                      # label block width for the row-side max
MAIN_W = 1536              # ACT-drained psum group width (3 banks)
DVE_W = 512                # DVE-drained psum group width (1 bank)
F16 = mybir.dt.float16
F32 = mybir.dt.float32
F8 = mybir.dt.float8e4

F8_MAX = 240.0                 # ml_dtypes.float8_e4m3 saturation
CASCADE_SCALES = (8.0, 1.0, 0.125)

_prog_cache = {}


def _groups(C):
    """(offset, width, engine) psum groups: five ACT groups of MAIN_W,
    remainder on DVE in chunks of <= DVE_W."""
    gs = []
    o = 0
    while o + MAIN_W <= C and len(gs) < 5:
        gs.append((o, MAIN_W, 'A'))
        o += MAIN_W
    while o < C:
        w = min(DVE_W, C - o)
        gs.append((o, w, 'D'))
        o += w
    return gs


def _chunks(w):
    out = []
    o = 0
    while o < w:
        cw = min(512, w - o)
        out.append((o, cw))
        o += cw
    return out


def _build(nrt: int, C: int):
    B = C // W
    groups = _groups(C)
    XA = nrt * 2 * P                   # unT / biasa free size
    # mainin: [ulT group0 (k-pair) | unT | ulT rest (k-major)]
    # biasin: [biasa | biasm (k-major, full C)]
    g0w = groups[0][1]
    main_x = 2 * C + XA
    bias_x = XA + 2 * C

    nc = bacc.Bacc("TRN2", target_bir_lowering=False, debug=False,
                   num_devices=N_CORES)
    main_in = nc.dram_tensor("mainin", [P, main_x], F8, kind="ExternalInput")
    bias_in = nc.dram_tensor("biasin", [4, bias_x], F8, kind="ExternalInput")
    rowout = nc.dram_tensor("rowout", [P, nrt * B], F16, kind="ExternalOutput")
    collout = nc.dram_tensor("collout", [P, C], F16, kind="ExternalOutput")

    with tile.TileContext(nc) as tc:
        with (
            tc.tile_pool(name="persist", bufs=1) as pp,
            tc.tile_pool(name="strip", bufs=3) as sp,
            tc.tile_pool(name="row", bufs=3) as rp,
        ):
            u_lT = pp.tile([P, 2, C], F8, tag="u_lT")
            u_nT = pp.tile([P, nrt, 2, P], F8, tag="u_nT")
            biasall = pp.tile([3, XA + 2 * C], F8, tag="biasall")
            biasa = biasall[:, :XA].rearrange("p (t k q) -> p t k q",
                                              t=nrt, k=2)
            biasm = biasall[:, XA:].rearrange("p (k c) -> p k c", k=2)
            coll = pp.tile([P, C], F16, tag="coll")

            # startup: the first matmul chunk needs only ulT cols [0:512),
            # the biases, and unT -- 3 small DMAs; the rest streams behind
            c0w = min(512, g0w)
            nc.sync.dma_start(u_nT[:, 0].rearrange("p k q -> p (k q)"),
                              main_in[:, 2 * g0w:2 * g0w + 2 * P])
            nc.sync.dma_start(u_lT[:, :, :c0w],
                              main_in[:, :2 * c0w].rearrange(
                                  "p (k c) -> p k c", k=2))
            nc.sync.dma_start(biasall[:], bias_in[:3, :])
            if g0w > c0w:
                nc.sync.dma_start(
                    u_lT[:, :, c0w:g0w],
                    main_in[:, 2 * c0w:2 * g0w].rearrange(
                        "p (k c) -> p k c", k=2))
            if nrt > 1:
                nc.sync.dma_start(
                    u_nT[:, 1:].rearrange("p t k q -> p (t k q)"),
                    main_in[:, 2 * g0w + 2 * P:2 * g0w + XA])
            # remaining label groups: k-major packing at offset 2*g0w+XA;
            # k0 block then k1 block, each C-g0w wide
            rest0 = 2 * g0w + XA
            rw = C - g0w
            for gi, (go, w, eng) in enumerate(groups):
                if gi == 0:
                    continue
                nc.sync.dma_start(
                    u_lT[:, :, go:go + w],
                    main_in[:, rest0:rest0 + 2 * rw].rearrange(
                        "p (k c) -> p k c", k=2)[:, :, go - g0w:go - g0w + w])

            def col_and_row(t, strips, last):
                """col running-max + row halves-max for tile t (lagged)."""
                strip = strips[t]
                src = coll if t == 0 else strip
                if t == 0 and last:
                    nc.sync.dma_start(collout[:], coll[:])
                if t > 0:
                    if not last:
                        nc.vector.tensor_max(coll[:], coll[:], strip[:])
                    else:
                        for go, w, eng in groups:
                            nc.vector.tensor_max(coll[:, go:go + w],
                                                 coll[:, go:go + w],
                                                 strip[:, go:go + w])
                            nc.sync.dma_start(collout[:, go:go + w],
                                              coll[:, go:go + w])
                rst = rp.tile([P, B], F16, tag="rst")
                if last:
                    # split halves so the first rowout DMA overlaps the rest
                    h = B // 2
                    nc.vector.tensor_max(rst[:, :h], src[:, 0:h],
                                         src[:, B:B + h])
                    nc.sync.dma_start(rowout[:, t * B:t * B + h],
                                      rst[:, :h])
                    nc.vector.tensor_max(rst[:, h:], src[:, h:B],
                                         src[:, B + h:2 * B])
                    nc.sync.dma_start(rowout[:, t * B + h:(t + 1) * B],
                                      rst[:, h:])
                else:
                    nc.vector.tensor_max(rst[:], src[:, 0:B],
                                         src[:, B:2 * B])
                    nc.sync.dma_start(rowout[:, t * B:(t + 1) * B], rst[:])

            strips = {}
            with (
                tc.tile_pool(name="psA", bufs=2, space="PSUM") as pga,
                tc.tile_pool(name="psD", bufs=2, space="PSUM") as pgd,
            ):
                for t in range(nrt):
                    strips[t] = strip = sp.tile([P, C], F16, tag="strip",
                                                name="strip")
                    for go, w, eng in groups:
                        if eng == 'A':
                            ptf = pga.tile([P, MAIN_W], F32, tag="dotA")
                        else:
                            ptf = pgd.tile([P, DVE_W], F32, tag="dotD")
                        pt = ptf[:, :w]
                        for c0, cw in _chunks(w):
                            nc.tensor.matmul(
                                pt[:, c0:c0 + cw],
                                u_nT[:, t, :, :],
                                u_lT[:, :, go + c0:go + c0 + cw],
                                start=True, stop=False,
                                perf_mode=mybir.MatmulPerfMode.DoubleRow)
                            nc.tensor.matmul(
                                pt[:, c0:c0 + cw],
                                biasa[:, t, :, :],
                                biasm[:, :, go + c0:go + c0 + cw],
                                start=False, stop=True,
                                perf_mode=mybir.MatmulPerfMode.DoubleRow)
                        dst = coll[:, go:go + w] if t == 0 else \
                            strip[:, go:go + w]
                        # tiles 0-1: DVE is underloaded (no/partial col
                        # pass) -- it absorbs trailing main-group drains;
                        # tiles 2-4: ACT absorbs the small dve groups to
                        # even out the steady-state load
                        if eng == 'A' and (
                                (t == 0 and go >= 3 * MAIN_W)
                                or (t == 1 and go >= 4 * MAIN_W)):
                            eng = 'D'
                        elif eng == 'D' and 2 <= t <= 4:
                            eng = 'A'
                        if eng == 'A':
                            nc.scalar.activation(
                                dst, pt[:],
                                mybir.ActivationFunctionType.Identity,
                                bias=0.0, scale=1.0)
                        else:
                            nc.vector.tensor_copy(dst, pt[:])
                    # DVE's heavy f16 passes lag one tile so the small DVE
                    # drains above never queue behind them (their psum
                    # release gates the next tile's matmuls)
                    if t > 0:
                        col_and_row(t - 1, strips, last=False)
                col_and_row(nrt - 1, strips, last=True)

    nc.compile()
    return nc


def _get_program(nrt, C):
    key = (nrt, C)
    if key not in _prog_cache:
        _prog_cache[key] = _build(nrt, C)
    return _prog_cache[key]


def _band_layout(sizes, nrt):
    """Lane bands: segment s gets L_s = ceil(size_s/nrt) lanes."""
    L = [-(-int(s) // nrt) if s > 0 else 0 for s in sizes]
    B = np.concatenate([[0], np.cumsum(L)]).astype(np.int64)
    return B, L


def _cascade(v, fdt):
    """Split v into 3 fp8 rows with per-row scales s_i so that
    sum_i s_i * row_i ~= v (residual cascade, |err| <= ~0.25). The scales
    ride the opposing operand's constant rows (exact powers of two).
    Handles |v| up to ~1900 (|u|^2 tails, e.g. self-paired edges)."""
    r = v.astype(np.float32)
    rows = []
    for s in CASCADE_SCALES:
        q = np.clip(r / s, -F8_MAX, F8_MAX).astype(fdt)
        rows.append(q)
        r = r - s * q.astype(np.float32)
    return np.stack(rows)          # [3, n]


def kernel(h, node_edge, node_batch, label_edge, label_batch):
    h = np.asarray(h)
    ne = np.asarray(node_edge).astype(np.int64)
    nb = np.asarray(node_batch).astype(np.int64)
    le = np.asarray(label_edge).astype(np.int64)
    lb = np.asarray(label_batch).astype(np.int64)
    fdt = mybir.dt.np(F8)

    cn = np.bincount(nb, minlength=GN).astype(np.int64)
    cl = np.bincount(lb, minlength=GL).astype(np.int64)
    nb_off = np.concatenate([[0], np.cumsum(cn)])
    lb_off = np.concatenate([[0], np.cumsum(cl)])

    # ---- label columns: member-major W-blocks per segment ------------------
    bg = -(-cl // W)                       # blocks per segment
    b_off = np.concatenate([[0], np.cumsum(bg)])
    B = int(b_off[-1])
    C = B * W

    col_edge = np.zeros(C, np.int64)       # strip col -> label edge index
    col_valid = np.zeros(C, bool)          # first occurrence of a label edge
    for g in range(GL):
        n_g = int(cl[g])
        if n_g == 0:
            continue
        bgg = int(bg[g])
        for m in range(W):
            k = np.arange(bgg)
            j = k + m * bgg                # segment-local label col
            valid = j < n_g
            jj = np.where(valid, j, k)     # dup member-0 col when past end
            col_edge[m * B + b_off[g] + k] = lb_off[g] + jj
            col_valid[m * B + b_off[g] + k] = valid

    hf = h.astype(np.float32)
    u_l = hf[le[0][col_edge]] + hf[le[1][col_edge]]            # [C, 256]
    bq = u_l.astype(fdt)                                       # quantized b
    bl2 = (bq.astype(np.float32) ** 2).sum(axis=1)             # |b|^2
    ulT = np.ascontiguousarray(
        bq.T.reshape(2, P, C).transpose(1, 0, 2))              # [P, 2, C]
    casb = _cascade(-bl2, fdt)                                 # [3, C]
    sc = np.array(CASCADE_SCALES, np.float32)
    bm = np.zeros((3, 2, C), fdt)
    bm[:, 0, :] = np.broadcast_to(sc[:, None], (3, C)).astype(fdt)
    bm[:, 1, :] = casb

    # device input packing (see _build): mainin = [ulT g0 k-pair | unT |
    # ulT rest k-major]; biasin = [biasa | biasm k-major]
    groups = _groups(C)
    g0w = groups[0][1]

    # ---- node rows: per-core lane bands over 8 segments --------------------
    core_sizes = cn.reshape(N_CORES, 8)
    nrt = max(1, int(-(-core_sizes.sum(1).max() // P)))
    while max(sum(-(-int(s) // nrt) for s in core_sizes[c] if s > 0)
              for c in range(N_CORES)) > P:
        nrt += 1
    nrows = nrt * P
    XA = nrt * 2 * P

    g0c0 = min(512, g0w)
    ul_part = np.concatenate([
        ulT[:, :, :g0c0].reshape(P, -1),         # g0 first chunk (k-major)
        ulT[:, :, g0c0:g0w].reshape(P, -1),      # rest of g0 (k-major)
        np.zeros((P, XA), fdt),                  # unT slot (filled per core)
        ulT[:, :, g0w:].reshape(P, -1),          # k-major rest
    ], axis=1)
    bm_flat = np.ascontiguousarray(bm.reshape(3, 2 * C))

    in_maps = []
    band_info = []
    for c in range(N_CORES):
        Bo, L = _band_layout(core_sizes[c], nrt)
        assert Bo[-1] <= P
        slot = np.zeros(nrows, np.int64)
        slot[:] = min(int(nb_off[8 * c]), ne.shape[1] - 1)
        for s in range(8):
            g = 8 * c + s
            n_g = int(cn[g])
            if n_g == 0:
                continue
            lanes_all = np.arange(Bo[s], Bo[s + 1])
            for tt in range(nrt):
                slot[tt * P + lanes_all] = nb_off[g]   # seg dup default
            j = np.arange(n_g)
            lanes = Bo[s] + j // nrt
            ts = j % nrt
            slot[ts * P + lanes] = nb_off[g] + j
        u_n = hf[ne[0][slot]] + hf[ne[1][slot]]                 # [nrows, 256]
        aq = (2.0 * u_n).astype(fdt)                            # quantized a
        an2 = ((aq.astype(np.float32) ** 2).sum(axis=1) * 0.25)
        # unT layout: [p(K%128), t, k, row]
        a = aq.reshape(nrt, P, 2, P)         # [t, row, k, p]
        unT = np.ascontiguousarray(a.transpose(3, 0, 2, 1).reshape(P, -1))
        casa = _cascade(-an2, fdt)                              # [3, nrows]
        ba = np.zeros((3, nrt, 2, P), fdt)
        ba[:, :, 0, :] = casa.reshape(3, nrt, P)
        ba[:, :, 1, :] = np.broadcast_to(
            sc[:, None, None], (3, nrt, P)).astype(fdt)
        main_arr = ul_part.copy()
        main_arr[:, 2 * g0w:2 * g0w + XA] = unT
        bias_arr = np.zeros((4, XA + 2 * C), fdt)
        bias_arr[:3, :XA] = ba.reshape(3, -1)
        bias_arr[:3, XA:] = bm_flat
        in_maps.append({
            "mainin": main_arr,
            "biasin": bias_arr,
        })
        band_info.append((Bo, L))

    nc = _get_program(nrt, C)
    res = run_bass_kernel_spmd(nc, in_maps, core_ids=list(range(N_CORES)))

    # ---- host unpack -------------------------------------------------------
    out_n = np.zeros((GN, GL), np.float64)
    out_l = np.zeros((GN, GL), np.float64)
    ridx = (b_off[:-1]).clip(0, max(B - 1, 0))
    for c in range(N_CORES):
        r = res.results[c]
        rowe = r["rowout"].astype(np.float64).reshape(P, nrt, B)
        colle = r["collout"].astype(np.float64)                 # [128, C]
        Bo, L = band_info[c]
        for s in range(8):
            g = 8 * c + s
            n_g = int(cn[g])
            if n_g == 0:
                continue
            j = np.arange(n_g)
            lanes = Bo[s] + j // nrt
            ts = j % nrt
            blk = rowe[lanes, ts, :]                            # [n_g, B]
            segmax = np.maximum.reduceat(blk, ridx, axis=1)
            d = 0.5 * np.sqrt(np.maximum(-segmax, 0.0))
            row_mean = -d.mean(axis=0)
            row_mean[cl == 0] = 0.0
            out_n[g] = row_mean

            ecol = colle[Bo[s]:Bo[s] + L[s], :].max(axis=0)     # [C]
            dcol = 0.5 * np.sqrt(np.maximum(-ecol, 0.0))
            sums = _seg_col_sums(dcol, col_valid, bg)
            col_mean = -(sums / np.maximum(cl, 1))
            col_mean[cl == 0] = 0.0
            out_l[g] = col_mean

    return ((out_n + out_l) * 0.5).astype(np.float32)


def _seg_col_sums(dcol, col_valid, bg):
    """Sum d over valid strip columns, grouped by label segment.

    Strip col m*B + (b_off[g] + k) belongs to segment g.
    """
    seg_of_block = np.repeat(np.arange(GL), bg.astype(np.int64))   # [B]
    seg_of_col = np.tile(seg_of_block, W)                          # [C]
    w = np.where(col_valid, dcol, 0.0)
    return np.bincount(seg_of_col, weights=w, minlength=GL)


# revision 20
# speedup vs baseline: 1.0383x; 1.0383x over previous
"""Trainium2 Bass kernel for nn_Explainer segment_reduce (cdist + bidirectional
segment max/mean) on 8 NeuronCores.

Math (reference):
    ef_n = (h[ne0] + h[ne1])/2, ef_l = (h[le0] + h[le1])/2
    M = -cdist(ef_n, ef_l)                      # [En, El]
    out_n = seg_mean_rows(seg_max_cols(M))      # [Gn, Gl]
    out_l = seg_mean_cols(seg_max_rows(M))      # [Gn, Gl]
    out = (out_n + out_l)/2

Device computes strip = a.b - |b|^2 - |a|^2/4 = -|u_n - u_l|^2 = -4d^2 with
a = 2u_n, b = u_l (fp8): the dot via fp8 DoubleRow matmuls (K=256), BOTH
norm terms via one extra fp8 K=6 DoubleRow matmul whose rows carry a
scale-cascade fp8 decomposition (scales 8, 1, 1/8 on the opposing operand's
constant rows; |err| <= ~0.25) of -|a_r|^2/4 (stationary side) and -|b_c|^2
(moving side). All segment reductions are then plain MAX on complete
values; host maps back via d = 0.5*sqrt(-v).

Sharding: core c owns node segments [8c, 8c+8) in per-segment lane bands
(segment s -> lanes [B_s, B_s+L_s), row-tiles t in [0, nrt)); dummy slots
duplicate the segment's first row (can't win a max). Label columns
replicated; each label segment padded to a multiple of W=2 with duplicate
edges, laid out MEMBER-MAJOR: member m of block b sits at column m*B + b,
so the row-side block max is a single unit-stride tensor_max of the two
C/2 halves (full 2x DVE rate).

Engine split per tile: PE fills psum (main + bias matmul per 512-chunk);
Scalar drains five [P,1536] psum groups to the f16 strip; DVE drains the
final 544 cols from its own small psum pool (own ring -> a full tile of
slack before its release gates PE), then does the full-width col running
max and the halves row max, lagged one tile so the small drains never
queue behind them. Host: fold pair-maxes per label segment + sqrt + means;
band-collapse the col accumulator + sqrt + masked means; assemble [64,64].
"""
import numpy as np

import concourse.bacc as bacc
import concourse.tile as tile
import concourse.mybir as mybir
from concourse.bass_utils import run_bass_kernel_spmd

P = 128
N_CORES = 8
GN = GL = 64
D = 256
W = 2                      # label block width for the row-side max
MAIN_W = 1536              # ACT-drained psum group width (3 banks)
DVE_W = 512                # DVE-drained psum group width (1 bank)
F16 = mybir.dt.float16
F32 = mybir.dt.float32
F8 = mybir.dt.float8e4

F8_MAX = 240.0                 # ml_dtypes.float8_e4m3 saturation
CASCADE_SCALES = (8.0, 1.0, 0.125)

_prog_cache = {}


def _groups(C):
    """(offset, width, engine) psum groups: five ACT groups of MAIN_W,
    remainder on DVE in chunks of <= DVE_W."""
    gs = []
    o = 0
    while o + MAIN_W <= C and len(gs) < 5:
        gs.append((o, MAIN_W, 'A'))
        o += MAIN_W
    while o < C:
        w = min(DVE_W, C - o)
        gs.append((o, w, 'D'))
        o += w
    return gs


def _chunks(w):
    out = []
    o = 0
    while o < w:
        cw = min(512, w - o)
        out.append((o, cw))
        o += cw
    return out


def _build(nrt: int, C: int):
    B = C // W
    groups = _groups(C)
    XA = nrt * 2 * P                   # unT / biasa free size
    # mainin: [ulT group0 (k-pair) | unT | ulT rest (k-major)]
    # biasin: [biasa | biasm (k-major, full C)]
    g0w = groups[0][1]
    main_x = 2 * C + XA
    bias_x = XA + 2 * C

    nc = bacc.Bacc("TRN2", target_bir_lowering=False, debug=False,
                   num_devices=N_CORES)
    main_in = nc.dram_tensor("mainin", [P, main_x], F8, kind="ExternalInput")
    bias_in = nc.dram_tensor("biasin", [4, bias_x], F8, kind="ExternalInput")
    rowout = nc.dram_tensor("rowout", [P, nrt * B], F16, kind="ExternalOutput")
    collout = nc.dram_tensor("collout", [P, C], F16, kind="ExternalOutput")

    with tile.TileContext(nc) as tc:
        with (
            tc.tile_pool(name="persist", bufs=1) as pp,
            tc.tile_pool(name="strip", bufs=3) as sp,
            tc.tile_pool(name="row", bufs=3) as rp,
        ):
            u_lT = pp.tile([P, 2, C], F8, tag="u_lT")
            u_nT = pp.tile([P, nrt, 2, P], F8, tag="u_nT")
            biasall = pp.tile([3, XA + 2 * C], F8, tag="biasall")
            biasa = biasall[:, :XA].rearrange("p (t k q) -> p t k q",
                                              t=nrt, k=2)
            biasm = biasall[:, XA:].rearrange("p (k c) -> p k c", k=2)
            coll = pp.tile([P, C], F16, tag="coll")

            # startup: the first matmul chunk needs only ulT cols [0:512),
            # the biases, and unT -- 3 small DMAs; the rest streams behind
            c0w = min(512, g0w)
            nc.sync.dma_start(u_nT[:, 0].rearrange("p k q -> p (k q)"),
                              main_in[:, 2 * g0w:2 * g0w + 2 * P])
            nc.sync.dma_start(u_lT[:, :, :c0w],
                              main_in[:, :2 * c0w].rearrange(
                                  "p (k c) -> p k c", k=2))
            nc.sync.dma_start(biasall[:], bias_in[:3, :])
            if g0w > c0w:
                nc.sync.dma_start(
                    u_lT[:, :, c0w:g0w],
                    main_in[:, 2 * c0w:2 * g0w].rearrange(
                        "p (k c) -> p k c", k=2))
            if nrt > 1:
                nc.sync.dma_start(
                    u_nT[:, 1:].rearrange("p t k q -> p (t k q)"),
                    main_in[:, 2 * g0w + 2 * P:2 * g0w + XA])
            # remaining label groups: k-major packing at offset 2*g0w+XA;
            # k0 block then k1 block, each C-g0w wide
            rest0 = 2 * g0w + XA
            rw = C - g0w
            for gi, (go, w, eng) in enumerate(groups):
                if gi == 0:
                    continue
                nc.sync.dma_start(
                    u_lT[:, :, go:go + w],
                    main_in[:, rest0:rest0 + 2 * rw].rearrange(
                        "p (k c) -> p k c", k=2)[:, :, go - g0w:go - g0w + w])

            def col_and_row(t, strips, last):
                """col running-max + row halves-max for tile t (lagged)."""
                strip = strips[t]
                src = coll if t == 0 else strip
                if t == 0 and last:
                    nc.sync.dma_start(collout[:], coll[:])
                rst = rp.tile([P, B], F16, tag="rst")
                if last:
                    # rst halves first so rowout DMAs clear the wire while
                    # DVE walks the per-group col updates
                    h = B // 2
                    nc.vector.tensor_max(rst[:, :h], src[:, 0:h],
                                         src[:, B:B + h])
                    nc.sync.dma_start(rowout[:, t * B:t * B + h],
                                      rst[:, :h])
                    nc.vector.tensor_max(rst[:, h:], src[:, h:B],
                                         src[:, B + h:2 * B])
                    nc.sync.dma_start(rowout[:, t * B + h:(t + 1) * B],
                                      rst[:, h:])
                else:
                    nc.vector.tensor_max(rst[:], src[:, 0:B],
                                         src[:, B:2 * B])
                    nc.sync.dma_start(rowout[:, t * B:(t + 1) * B], rst[:])
                if t > 0:
                    if not last:
                        nc.vector.tensor_max(coll[:], coll[:], strip[:])
                    else:
                        for go, w, eng in groups:
                            nc.vector.tensor_max(coll[:, go:go + w],
                                                 coll[:, go:go + w],
                                                 strip[:, go:go + w])
                            nc.sync.dma_start(collout[:, go:go + w],
                                              coll[:, go:go + w])

            strips = {}
            with (
                tc.tile_pool(name="psA", bufs=2, space="PSUM") as pga,
                tc.tile_pool(name="psD", bufs=2, space="PSUM") as pgd,
            ):
                for t in range(nrt):
                    strips[t] = strip = sp.tile([P, C], F16, tag="strip",
                                                name="strip")
                    for go, w, eng in groups:
                        if eng == 'A':
                            ptf = pga.tile([P, MAIN_W], F32, tag="dotA")
                        else:
                            ptf = pgd.tile([P, DVE_W], F32, tag="dotD")
                        pt = ptf[:, :w]
                        for c0, cw in _chunks(w):
                            nc.tensor.matmul(
                                pt[:, c0:c0 + cw],
                                u_nT[:, t, :, :],
                                u_lT[:, :, go + c0:go + c0 + cw],
                                start=True, stop=False,
                                perf_mode=mybir.MatmulPerfMode.DoubleRow)
                            nc.tensor.matmul(
                                pt[:, c0:c0 + cw],
                                biasa[:, t, :, :],
                                biasm[:, :, go + c0:go + c0 + cw],
                                start=False, stop=True,
                                perf_mode=mybir.MatmulPerfMode.DoubleRow)
                        dst = coll[:, go:go + w] if t == 0 else \
                            strip[:, go:go + w]
                        # tiles 0-1: DVE is underloaded (no/partial col
                        # pass) -- it absorbs trailing main-group drains;
                        # tiles 2-4: ACT absorbs the small dve groups to
                        # even out the steady-state load
                        if eng == 'A' and (
                                (t == 0 and go >= 3 * MAIN_W)
                                or (t == 1 and go >= 4 * MAIN_W)):
                            eng = 'D'
                        if eng == 'A':
                            nc.scalar.activation(
                                dst, pt[:],
                                mybir.ActivationFunctionType.Identity,
                                bias=0.0, scale=1.0)
                        else:
                            nc.vector.tensor_copy(dst, pt[:])
                    # DVE's heavy f16 passes lag one tile so the small DVE
                    # drains above never queue behind them (their psum
                    # release gates the next tile's matmuls)
                    if t > 0:
                        col_and_row(t - 1, strips, last=False)
                col_and_row(nrt - 1, strips, last=True)

    nc.compile()
    return nc


def _get_program(nrt, C):
    key = (nrt, C)
    if key not in _prog_cache:
        _prog_cache[key] = _build(nrt, C)
    return _prog_cache[key]


def _band_layout(sizes, nrt):
    """Lane bands: segment s gets L_s = ceil(size_s/nrt) lanes."""
    L = [-(-int(s) // nrt) if s > 0 else 0 for s in sizes]
    B = np.concatenate([[0], np.cumsum(L)]).astype(np.int64)
    return B, L


def _cascade(v, fdt):
    """Split v into 3 fp8 rows with per-row scales s_i so that
    sum_i s_i * row_i ~= v (residual cascade, |err| <= ~0.25). The scales
    ride the opposing operand's constant rows (exact powers of two).
    Handles |v| up to ~1900 (|u|^2 tails, e.g. self-paired edges)."""
    r = v.astype(np.float32)
    rows = []
    for s in CASCADE_SCALES:
        q = np.clip(r / s, -F8_MAX, F8_MAX).astype(fdt)
        rows.append(q)
        r = r - s * q.astype(np.float32)
    return np.stack(rows)          # [3, n]


def kernel(h, node_edge, node_batch, label_edge, label_batch):
    h = np.asarray(h)
    ne = np.asarray(node_edge).astype(np.int64)
    nb = np.asarray(node_batch).astype(np.int64)
    le = np.asarray(label_edge).astype(np.int64)
    lb = np.asarray(label_batch).astype(np.int64)
    fdt = mybir.dt.np(F8)

    cn = np.bincount(nb, minlength=GN).astype(np.int64)
    cl = np.bincount(lb, minlength=GL).astype(np.int64)
    nb_off = np.concatenate([[0], np.cumsum(cn)])
    lb_off = np.concatenate([[0], np.cumsum(cl)])

    # ---- label columns: member-major W-blocks per segment ------------------
    bg = -(-cl // W)                       # blocks per segment
    b_off = np.concatenate([[0], np.cumsum(bg)])
    B = int(b_off[-1])
    C = B * W

    col_edge = np.zeros(C, np.int64)       # strip col -> label edge index
    col_valid = np.zeros(C, bool)          # first occurrence of a label edge
    for g in range(GL):
        n_g = int(cl[g])
        if n_g == 0:
            continue
        bgg = int(bg[g])
        for m in range(W):
            k = np.arange(bgg)
            j = k + m * bgg                # segment-local label col
            valid = j < n_g
            jj = np.where(valid, j, k)     # dup member-0 col when past end
            col_edge[m * B + b_off[g] + k] = lb_off[g] + jj
            col_valid[m * B + b_off[g] + k] = valid

    hf = h.astype(np.float32)
    u_l = hf[le[0][col_edge]] + hf[le[1][col_edge]]            # [C, 256]
    bq = u_l.astype(fdt)                                       # quantized b
    bl2 = (bq.astype(np.float32) ** 2).sum(axis=1)             # |b|^2
    ulT = np.ascontiguousarray(
        bq.T.reshape(2, P, C).transpose(1, 0, 2))              # [P, 2, C]
    casb = _cascade(-bl2, fdt)                                 # [3, C]
    sc = np.array(CASCADE_SCALES, np.float32)
    bm = np.zeros((3, 2, C), fdt)
    bm[:, 0, :] = np.broadcast_to(sc[:, None], (3, C)).astype(fdt)
    bm[:, 1, :] = casb

    # device input packing (see _build): mainin = [ulT g0 k-pair | unT |
    # ulT rest k-major]; biasin = [biasa | biasm k-major]
    groups = _groups(C)
    g0w = groups[0][1]

    # ---- node rows: per-core lane bands over 8 segments --------------------
    core_sizes = cn.reshape(N_CORES, 8)
    nrt = max(1, int(-(-core_sizes.sum(1).max() // P)))
    while max(sum(-(-int(s) // nrt) for s in core_sizes[c] if s > 0)
              for c in range(N_CORES)) > P:
        nrt += 1
    nrows = nrt * P
    XA = nrt * 2 * P

    g0c0 = min(512, g0w)
    ul_part = np.concatenate([
        ulT[:, :, :g0c0].reshape(P, -1),         # g0 first chunk (k-major)
        ulT[:, :, g0c0:g0w].reshape(P, -1),      # rest of g0 (k-major)
        np.zeros((P, XA), fdt),                  # unT slot (filled per core)
        ulT[:, :, g0w:].reshape(P, -1),          # k-major rest
    ], axis=1)
    bm_flat = np.ascontiguousarray(bm.reshape(3, 2 * C))

    in_maps = []
    band_info = []
    for c in range(N_CORES):
        Bo, L = _band_layout(core_sizes[c], nrt)
        assert Bo[-1] <= P
        slot = np.zeros(nrows, np.int64)
        slot[:] = min(int(nb_off[8 * c]), ne.shape[1] - 1)
        for s in range(8):
            g = 8 * c + s
            n_g = int(cn[g])
            if n_g == 0:
                continue
            lanes_all = np.arange(Bo[s], Bo[s + 1])
            for tt in range(nrt):
                slot[tt * P + lanes_all] = nb_off[g]   # seg dup default
            j = np.arange(n_g)
            lanes = Bo[s] + j // nrt
            ts = j % nrt
            slot[ts * P + lanes] = nb_off[g] + j
        u_n = hf[ne[0][slot]] + hf[ne[1][slot]]                 # [nrows, 256]
        aq = (2.0 * u_n).astype(fdt)                            # quantized a
        an2 = ((aq.astype(np.float32) ** 2).sum(axis=1) * 0.25)
        # unT layout: [p(K%128), t, k, row]
        a = aq.reshape(nrt, P, 2, P)         # [t, row, k, p]
        unT = np.ascontiguousarray(a.transpose(3, 0, 2, 1).reshape(P, -1))
        casa = _cascade(-an2, fdt)                              # [3, nrows]
        ba = np.zeros((3, nrt, 2, P), fdt)
        ba[:, :, 0, :] = casa.reshape(3, nrt, P)
        ba[:, :, 1, :] = np.broadcast_to(
            sc[:, None, None], (3, nrt, P)).astype(fdt)
        main_arr = ul_part.copy()
        main_arr[:, 2 * g0w:2 * g0w + XA] = unT
        bias_arr = np.zeros((4, XA + 2 * C), fdt)
        bias_arr[:3, :XA] = ba.reshape(3, -1)
        bias_arr[:3, XA:] = bm_flat
        in_maps.append({
            "mainin": main_arr,
            "biasin": bias_arr,
        })
        band_info.append((Bo, L))

    nc = _get_program(nrt, C)
    res = run_bass_kernel_spmd(nc, in_maps, core_ids=list(range(N_CORES)))

    # ---- host unpack -------------------------------------------------------
    out_n = np.zeros((GN, GL), np.float64)
    out_l = np.zeros((GN, GL), np.float64)
    ridx = (b_off[:-1]).clip(0, max(B - 1, 0))
    for c in range(N_CORES):
        r = res.results[c]
        rowe = r["rowout"].astype(np.float64).reshape(P, nrt, B)
        colle = r["collout"].astype(np.float64)                 # [128, C]
        Bo, L = band_info[c]
        for s in range(8):
            g = 8 * c + s
            n_g = int(cn[g])
            if n_g == 0:
                continue
            j = np.arange(n_g)
            lanes = Bo[s] + j // nrt
            ts = j % nrt
            blk = rowe[lanes, ts, :]                            # [n_g, B]
            segmax = np.maximum.reduceat(blk, ridx, axis=1)
            d = 0.5 * np.sqrt(np.maximum(-segmax, 0.0))
            row_mean = -d.mean(axis=0)
            row_mean[cl == 0] = 0.0
            out_n[g] = row_mean

            ecol = colle[Bo[s]:Bo[s] + L[s], :].max(axis=0)     # [C]
            dcol = 0.5 * np.sqrt(np.maximum(-ecol, 0.0))
            sums = _seg_col_sums(dcol, col_valid, bg)
            col_mean = -(sums / np.maximum(cl, 1))
            col_mean[cl == 0] = 0.0
            out_l[g] = col_mean

    return ((out_n + out_l) * 0.5).astype(np.float32)


def _seg_col_sums(dcol, col_valid, bg):
    """Sum d over valid strip columns, grouped by label segment.

    Strip col m*B + (b_off[g] + k) belongs to segment g.
    """
    seg_of_block = np.repeat(np.arange(GL), bg.astype(np.int64))   # [B]
    seg_of_col = np.tile(seg_of_block, W)                          # [C]
    w = np.where(col_valid, dcol, 0.0)
    return np.bincount(seg_of_col, weights=w, minlength=GL)
